# revision 3
# baseline (speedup 1.0000x reference)
"""Causal self-attention (GQA, partial RoPE, qk rms-norm, logit softcap) on 8 trn2 cores.

Sharding: 8 cores = batch(2) x kv_head(4). Each core computes, for its (b, h):
  - q/k/v projections for its 4 q-heads / 1 kv-head (x @ W.T slices)
  - rms-norm, partial rope, q_gain, causal softcapped attention
  - partial output projection against Wproj columns [512h:512h+512]
Host sums the 4 partials per batch.

v2 changes vs the f32r baseline (433us):
  - All heavy matmuls run in bf16. Sustained fp32-mode PE activity tripped the
    hardware power limiter (ham trace: 139us at 50% util cap, ~70us lost).
  - q/k transposes into [f, s] layout go through the DMA XBAR (16x128 tiles)
    instead of PE identity-matmuls, freeing PE time and a PSUM bank.
  - softmax denominator: Pool engine accumulates the probability tiles into an
    f32 SBUF accumulator; one ones-matmul per (c, g) replaces one per kb
    (saves ~33us of PE moving time).
  - rms statistics come free from scalar Square + accum_out (frees DVE).
  - rope tables are pre-scaled by gain*rstd so the scale pass disappears.
  - single merged pool context (no inter-phase barrier), x s-tile 0 DMA'd
    before the weights, exact per-kb causal offsets (off = 128*r).
  - Wproj output projection of chunk c-1 interleaves into chunk c's attention
    groups to fill scalar-bound stretches.
"""
import math
import numpy as np
from contextlib import ExitStack

import ml_dtypes
import concourse.bass as bass
import concourse.tile as tile
from concourse import bacc, mybir
from concourse.bass_utils import run_bass_kernel_spmd
from concourse.alu_op_type import AluOpType

F32 = mybir.dt.float32
F32R = mybir.dt.float32r
BF16 = mybir.dt.bfloat16

B = 2
S = 2048
D = 2048
H = 16
HKV = 4
HD = 128
G = 4  # q heads per kv head (= heads per core)
ROPE = 32
HALF = ROPE // 2  # 16
ROPE_BASE = 10000.0
CAP = 30.0
EPS = float(np.finfo(np.float32).eps)
NST = S // 128  # 16 s-tiles
NCH = S // 512  # 4 sq chunks
NDT = D // 128  # 16 d k-tiles
FQKV = G * HD + 2 * HD  # 768

_CACHE = {}


def _build():
    nc = bacc.Bacc("TRN2", target_bir_lowering=False, debug=False)

    xT = nc.dram_tensor("xT", [D, S], BF16, kind="ExternalInput").ap()
    wqkv = nc.dram_tensor("wqkv", [D, FQKV], BF16, kind="ExternalInput").ap()
    wpT = nc.dram_tensor("wpT", [G * HD, D], BF16, kind="ExternalInput").ap()
    gains = nc.dram_tensor("gains", [128, G], F32, kind="ExternalInput").ap()
    cos4 = nc.dram_tensor("cos4", [S, G * HALF], F32, kind="ExternalInput").ap()
    sin4 = nc.dram_tensor("sin4", [S, G * HALF], F32, kind="ExternalInput").ap()
    out = nc.dram_tensor("out", [S, D], F32, kind="ExternalOutput").ap()

    xT_r = xT.rearrange("(dt p) s -> p dt s", p=128)       # [128, 16, 2048]
    wqkv_r = wqkv.rearrange("(dt p) f -> p dt f", p=128)   # [128, 16, 768]
    wpT_r = wpT.rearrange("(g p) j -> p g j", p=128)       # [128, 4, 2048]
    cos_r = cos4.rearrange("(t p) f -> p t f", p=128)      # [128, 16, 64]
    sin_r = sin4.rearrange("(t p) f -> p t f", p=128)
    out_r = out.rearrange("(t p) j -> t p j", p=128)       # [16, 128, 2048]

    with tile.TileContext(nc) as tc:
        with ExitStack() as ctx:
            persist = ctx.enter_context(tc.tile_pool(name="persist", bufs=1))

            ones_f = persist.tile([128, 1], F32)
            nc.vector.memset(ones_f, 1.0)
            ones_col = persist.tile([128, 1], F32R)   # M=1 stationary for denom
            nc.vector.tensor_copy(ones_col, ones_f)
            ones_rowf = persist.tile([1, 128], F32)
            nc.vector.memset(ones_rowf, 1.0)
            ones_row = persist.tile([1, 128], F32R)   # K=1 stationary for broadcast
            nc.vector.tensor_copy(ones_row, ones_rowf)

            # diagonal-block 0/1 masks (r = kb - 4c in 0..3): valid iff sq >= r*128 + sk
            masks = persist.tile([128, 4, 512], BF16)
            mask_f = persist.tile([128, 512], F32)
            for r in range(4):
                nc.vector.memset(mask_f, 1.0)
                nc.gpsimd.affine_select(
                    out=mask_f, in_=mask_f, compare_op=AluOpType.is_ge,
                    fill=0.0, base=-128 * r, pattern=[[1, 512]], channel_multiplier=-1,
                )
                nc.vector.tensor_copy(masks[:, r, :], mask_f)

            eps_t = persist.tile([128, 1], F32)
            nc.vector.memset(eps_t, EPS)

            gains_sb = persist.tile([128, G], F32)
            nc.sync.dma_start(out=gains_sb, in_=gains)
            cos_all = persist.tile([128, NST, G * HALF], F32)
            sin_all = persist.tile([128, NST, G * HALF], F32)
            nc.sync.dma_start(out=cos_all, in_=cos_r)
            nc.sync.dma_start(out=sin_all, in_=sin_r)

            qT_all = persist.tile([128, G, S], BF16)   # [f, g, s]
            kT_all = persist.tile([128, S], BF16)      # [f, s]
            v_all = persist.tile([128, NST, HD], BF16)  # [sk within tile, st, f]
            yT_all = persist.tile([128, G, S], BF16)   # [f, g, s]
            wqkv_sb = persist.tile([128, NDT, FQKV], BF16)
            wpT_sb = persist.tile([128, G, D], BF16)

            xc_pool = ctx.enter_context(tc.tile_pool(name="xc", bufs=3))
            p1 = ctx.enter_context(tc.tile_pool(name="p1", bufs=2))
            p2 = ctx.enter_context(tc.tile_pool(name="p2", bufs=2))
            ps = ctx.enter_context(tc.tile_pool(name="ps", bufs=1, space="PSUM"))

            # x s-tile 0 first so the first matmul isn't gated on all weights
            xcs = []
            xc0 = xc_pool.tile([128, NDT, 128], BF16, tag="xc", name="xc0")
            nc.sync.dma_start(out=xc0, in_=xT_r[:, :, 0:128])
            xcs.append(xc0)
            for dt in range(NDT):
                nc.sync.dma_start(out=wqkv_sb[:, dt, :], in_=wqkv_r[:, dt, :])
            nc.sync.dma_start(out=wpT_sb, in_=wpT_r)

            def qkv_tile(st):
                xc = xcs[st]
                if st + 1 < NST:
                    nxt = xc_pool.tile([128, NDT, 128], BF16, tag="xc", name="xcn")
                    nc.sync.dma_start(
                        out=nxt, in_=xT_r[:, :, (st + 1) * 128:(st + 2) * 128])
                    xcs.append(nxt)

                psq = ps.tile([128, G * HD], F32, tag="mm512", bufs=3, name="psq")
                pskv = ps.tile([128, 2 * HD], F32, tag="kv", bufs=2, name="pskv")
                for dt in range(NDT):
                    nc.tensor.matmul(psq, xc[:, dt, :], wqkv_sb[:, dt, 0:G * HD],
                                     start=(dt == 0), stop=(dt == NDT - 1))
                for dt in range(NDT):
                    nc.tensor.matmul(pskv, xc[:, dt, :], wqkv_sb[:, dt, G * HD:FQKV],
                                     start=(dt == 0), stop=(dt == NDT - 1))

                # v: straight evacuation (no norm) on ScalarE
                nc.scalar.copy(v_all[:, st, :], pskv[:, HD:2 * HD])

                # rms statistics: Square with free-axis accumulation (ScalarE)
                psq_v = psq.rearrange("p (g d) -> p g d", g=G)
                ms = p1.tile([128, 5], F32, tag="ms")
                sq = p1.tile([128, HD], F32, tag="sq")
                for h in range(G):
                    nc.scalar.activation(sq, psq_v[:, h, :],
                                         mybir.ActivationFunctionType.Square,
                                         accum_out=ms[:, h:h + 1])
                nc.scalar.activation(sq, pskv[:, 0:HD],
                                     mybir.ActivationFunctionType.Square,
                                     accum_out=ms[:, 4:5])
                rstd = p1.tile([128, 5], F32, tag="rstd")
                nc.scalar.activation(rstd, ms, mybir.ActivationFunctionType.Sqrt,
                                     scale=1.0 / HD, bias=eps_t)
                nc.vector.reciprocal(rstd, rstd)
                gsc = p1.tile([128, G], F32, tag="gsc")
                nc.vector.tensor_mul(gsc, rstd[:, 0:4], gains_sb)

                cos_t = cos_all[:, st, :].rearrange("p (g d) -> p g d", g=G)
                sin_t = sin_all[:, st, :].rearrange("p (g d) -> p g d", g=G)

                # rope tables pre-scaled by gain*rstd (q) / rstd (k): the
                # per-head scale pass folds into the rotation itself
                csc = p1.tile([128, G, HALF], F32, tag="csc")
                ssc = p1.tile([128, G, HALF], F32, tag="ssc")
                for h in range(G):
                    nc.vector.tensor_scalar_mul(csc[:, h, :], cos_t[:, h, :],
                                                gsc[:, h:h + 1])
                    nc.vector.tensor_scalar_mul(ssc[:, h, :], sin_t[:, h, :],
                                                gsc[:, h:h + 1])

                q_sc = p1.tile([128, G, HD], BF16, tag="q_sc")
                t1 = p1.tile([128, G, HALF], F32, tag="t1")
                t2 = p1.tile([128, G, HALF], F32, tag="t2")
                nc.vector.tensor_mul(t1, psq_v[:, :, 0:HALF], csc)
                nc.vector.tensor_mul(t2, psq_v[:, :, HALF:ROPE], ssc)
                nc.vector.tensor_add(q_sc[:, :, 0:HALF], t1, t2)
                nc.vector.tensor_mul(t1, psq_v[:, :, HALF:ROPE], csc)
                nc.vector.tensor_mul(t2, psq_v[:, :, 0:HALF], ssc)
                nc.vector.tensor_sub(q_sc[:, :, HALF:ROPE], t1, t2)
                for h in range(G):
                    nc.vector.tensor_scalar_mul(q_sc[:, h, ROPE:HD],
                                                psq_v[:, h, ROPE:HD],
                                                gsc[:, h:h + 1])

                kcs = p1.tile([128, HALF], F32, tag="kcs")
                kss = p1.tile([128, HALF], F32, tag="kss")
                nc.vector.tensor_scalar_mul(kcs, cos_all[:, st, 0:HALF], rstd[:, 4:5])
                nc.vector.tensor_scalar_mul(kss, sin_all[:, st, 0:HALF], rstd[:, 4:5])
                k_sc = p1.tile([128, HD], BF16, tag="k_sc")
                kt1 = p1.tile([128, HALF], F32, tag="kt1")
                kt2 = p1.tile([128, HALF], F32, tag="kt2")
                nc.vector.tensor_mul(kt1, pskv[:, 0:HALF], kcs)
                nc.vector.tensor_mul(kt2, pskv[:, HALF:ROPE], kss)
                nc.vector.tensor_add(k_sc[:, 0:HALF], kt1, kt2)
                nc.vector.tensor_mul(kt1, pskv[:, HALF:ROPE], kcs)
                nc.vector.tensor_mul(kt2, pskv[:, 0:HALF], kss)
                nc.vector.tensor_sub(k_sc[:, HALF:ROPE], kt1, kt2)
                nc.vector.tensor_scalar_mul(k_sc[:, ROPE:HD], pskv[:, ROPE:HD],
                                            rstd[:, 4:5])

                # [s, f] -> [f, s] via the DMA XBAR (16x128 tiles, bf16)
                for h in range(G):
                    nc.sync.dma_start(out=qT_all[:, h, st * 128:(st + 1) * 128],
                                      in_=q_sc[:, h, :], transpose=True)
                nc.sync.dma_start(out=kT_all[:, st * 128:(st + 1) * 128],
                                  in_=k_sc, transpose=True)

            def attn_group(c, g):
                nkv = 4 * (c + 1)
                qT_c = qT_all[:, g, c * 512:(c + 1) * 512]
                ps_y = ps.tile([128, 512], F32, tag="yo", bufs=2, name="ps_y")
                acc = p2.tile([128, 512], F32R, tag="acc", name="acc")
                for kb in range(nkv):
                    r = kb - 4 * c
                    off = 128 * r if r > 0 else 0
                    ps_s = ps.tile([128, 512], F32, tag="mm512", bufs=3, name="ps_s")
                    nc.tensor.matmul(ps_s[:, off:512],
                                     kT_all[:, kb * 128:(kb + 1) * 128],
                                     qT_c[:, off:512], start=True, stop=True)
                    t = p2.tile([128, 512], F32, tag="t", name="t")
                    nc.scalar.activation(t[:, off:512], ps_s[:, off:512],
                                         mybir.ActivationFunctionType.Tanh,
                                         scale=1.0 / CAP)
                    p = p2.tile([128, 512], BF16, tag="p", bufs=3, name="p")
                    nc.scalar.activation(p[:, off:512], t[:, off:512],
                                         mybir.ActivationFunctionType.Exp,
                                         scale=CAP)
                    if r >= 0:
                        nc.vector.tensor_mul(p[:, off:512], p[:, off:512],
                                             masks[:, r, off:512])
                    # denominator partials accumulate on the (idle) Pool engine
                    if kb == 0:
                        nc.gpsimd.tensor_copy(acc, p)
                    else:
                        nc.gpsimd.tensor_add(acc[:, off:512], acc[:, off:512],
                                             p[:, off:512])
                    nc.tensor.matmul(ps_y[:, off:512], v_all[:, kb, :],
                                     p[:, off:512],
                                     start=(kb == 0), stop=(kb == nkv - 1))
                # denom -> [1,512] -> broadcast to 128 partitions -> recip
                ps_d = ps.tile([1, 512], F32, tag="dd", bufs=1, name="ps_d")
                nc.tensor.matmul(ps_d, ones_col, acc, start=True, stop=True)
                dn = p2.tile([1, 512], F32R, tag="dn", name="dn")
                nc.vector.tensor_copy(dn, ps_d)
                ps_b = ps.tile([128, 512], F32, tag="dd", bufs=1, name="ps_b")
                nc.tensor.matmul(ps_b, ones_row, dn, start=True, stop=True)
                recip = p2.tile([128, 512], F32, tag="recip", name="recip")
                nc.vector.reciprocal(recip, ps_b)
                nc.vector.tensor_mul(yT_all[:, g, c * 512:(c + 1) * 512], ps_y, recip)

            def proj_group(st, jc):
                ps_o = ps.tile([128, 512], F32, tag="yo", bufs=2, name="ps_o")
                for g in range(G):
                    nc.tensor.matmul(ps_o,
                                     yT_all[:, g, st * 128:(st + 1) * 128],
                                     wpT_sb[:, g, jc * 512:(jc + 1) * 512],
                                     start=(g == 0), stop=(g == G - 1))
                o_sb = p2.tile([128, 512], F32, tag="o_sb", bufs=3, name="o_sb")
                nc.vector.tensor_copy(o_sb, ps_o)
                nc.sync.dma_start(out=out_r[st][:, jc * 512:(jc + 1) * 512], in_=o_sb)

            for st in range(NST):
                qkv_tile(st)

            # chunk c-1's output projection interleaves into chunk c's
            # attention groups to fill the scalar-bound stretches
            pending = []
            for c in range(NCH):
                for g in range(G):
                    attn_group(c, g)
                    for _ in range(4):
                        if pending:
                            proj_group(*pending.pop(0))
                for st in range(4 * c, 4 * c + 4):
                    for jc in range(4):
                        pending.append((st, jc))
            while pending:
                proj_group(*pending.pop(0))

    nc.compile()
    return nc


def _host_prep(x, Wq, Wk, Wv, Wproj, q_gain):
    inv_freq = 1.0 / (ROPE_BASE ** (np.arange(0, ROPE, 2, dtype=np.float32) / ROPE))
    t = np.arange(S, dtype=np.float32)
    freqs = np.outer(t, inv_freq).astype(np.float32)  # [S, 16]
    cos = np.cos(freqs).astype(np.float32)
    sin = np.sin(freqs).astype(np.float32)
    cos4 = np.ascontiguousarray(np.tile(cos[:, None, :], (1, G, 1)).reshape(S, G * HALF))
    sin4 = np.ascontiguousarray(np.tile(sin[:, None, :], (1, G, 1)).reshape(S, G * HALF))

    xT = [np.ascontiguousarray(x[b].T).astype(ml_dtypes.bfloat16) for b in range(B)]

    in_maps = []
    for core in range(8):
        b, h = core // HKV, core % HKV
        wqkv = np.ascontiguousarray(
            np.concatenate(
                [Wq[512 * h:512 * h + 512].T,
                 Wk[128 * h:128 * h + 128].T,
                 Wv[128 * h:128 * h + 128].T], axis=1
            )
        ).astype(ml_dtypes.bfloat16)
        wpT = np.ascontiguousarray(
            Wproj[:, 512 * h:512 * h + 512].T).astype(ml_dtypes.bfloat16)
        gains = np.ascontiguousarray(
            np.broadcast_to((q_gain[G * h:G * h + G] / math.sqrt(HD)).astype(np.float32)[None, :],
                            (128, G))
        )
        in_maps.append({
            "xT": xT[b],
            "wqkv": wqkv,
            "wpT": wpT,
            "gains": gains,
            "cos4": cos4,
            "sin4": sin4,
        })
    return in_maps


def kernel(x, Wq, Wk, Wv, Wproj, q_gain, _trace=False):
    x = np.asarray(x, dtype=np.float32)
    Wq = np.asarray(Wq, dtype=np.float32)
    Wk = np.asarray(Wk, dtype=np.float32)
    Wv = np.asarray(Wv, dtype=np.float32)
    Wproj = np.asarray(Wproj, dtype=np.float32)
    q_gain = np.asarray(q_gain, dtype=np.float32)

    if "nc" not in _CACHE:
        _CACHE["nc"] = _build()
    nc = _CACHE["nc"]

    in_maps = _host_prep(x, Wq, Wk, Wv, Wproj, q_gain)
    res = run_bass_kernel_spmd(nc, in_maps, core_ids=list(range(8)), trace=_trace)

    out = np.empty((B, S, D), dtype=np.float32)
    for b in range(B):
        acc = np.zeros((S, D), dtype=np.float64)
        for h in range(HKV):
            acc += res.results[b * HKV + h]["out"]
        out[b] = acc.astype(np.float32)
    if _trace:
        return out, res
    return out


# revision 4
# speedup vs baseline: 1.1895x; 1.1895x over previous
"""Causal self-attention (GQA, partial RoPE, qk rms-norm, logit softcap) on 8 trn2 cores.

Sharding: 8 cores = batch(2) x kv_head(4). Each core computes, for its (b, h):
  - q/k/v projections for its 4 q-heads / 1 kv-head (x @ W.T slices)
  - rms-norm, partial rope, q_gain, causal softcapped attention
  - partial output projection against Wproj columns [512h:512h+512]
Host sums the 4 partials per batch.

All heavy matmuls run in bf16 (halves DMA/SBUF traffic vs f32r at the same
1 cycle/row PE issue rate). Scores are computed transposed (sT[sk, sq]) so
attention @ V needs no transpose of the probabilities; the softmax denominator
comes from a per-kb M=1 ones-matmul accumulated in PSUM. q/k transposes into
[f, s] layout are PE identity-matmuls (bf16), deferred one s-tile so they never
stall on the rope chain. One merged pool context: qkv tiles, then attention
chunks with the previous chunk's output projection interleaved to fill
scalar-bound stretches. rms statistics come free from scalar Square+accum_out.
"""
import math
import numpy as np
from contextlib import ExitStack

import ml_dtypes
import concourse.bass as bass
import concourse.tile as tile
from concourse import bacc, mybir
from concourse.bass_utils import run_bass_kernel_spmd
from concourse.masks import make_identity
from concourse.alu_op_type import AluOpType

F32 = mybir.dt.float32
F32R = mybir.dt.float32r
BF16 = mybir.dt.bfloat16

B = 2
S = 2048
D = 2048
H = 16
HKV = 4
HD = 128
G = 4  # q heads per kv head (= heads per core)
ROPE = 32
HALF = ROPE // 2  # 16
ROPE_BASE = 10000.0
CAP = 30.0
EPS = float(np.finfo(np.float32).eps)
NST = S // 128  # 16 s-tiles
NCH = S // 512  # 4 sq chunks
NDT = D // 128  # 16 d k-tiles
FQKV = G * HD + 2 * HD  # 768

_CACHE = {}


def _build():
    nc = bacc.Bacc("TRN2", target_bir_lowering=False, debug=False)

    xT = nc.dram_tensor("xT", [D, S], BF16, kind="ExternalInput").ap()
    wqkv = nc.dram_tensor("wqkv", [D, FQKV], BF16, kind="ExternalInput").ap()
    wpT = nc.dram_tensor("wpT", [G * HD, D], BF16, kind="ExternalInput").ap()
    gains = nc.dram_tensor("gains", [128, G], F32, kind="ExternalInput").ap()
    cos4 = nc.dram_tensor("cos4", [S, G * HALF], F32, kind="ExternalInput").ap()
    sin4 = nc.dram_tensor("sin4", [S, G * HALF], F32, kind="ExternalInput").ap()
    out = nc.dram_tensor("out", [S, D], F32, kind="ExternalOutput").ap()

    xT_r = xT.rearrange("(dt p) s -> p dt s", p=128)       # [128, 16, 2048]
    wqkv_r = wqkv.rearrange("(dt p) f -> p dt f", p=128)   # [128, 16, 768]
    wpT_r = wpT.rearrange("(g p) j -> p g j", p=128)       # [128, 4, 2048]
    cos_r = cos4.rearrange("(t p) f -> p t f", p=128)      # [128, 16, 64]
    sin_r = sin4.rearrange("(t p) f -> p t f", p=128)
    out_r = out.rearrange("(t p) j -> t p j", p=128)       # [16, 128, 2048]

    with tile.TileContext(nc) as tc:
        with ExitStack() as ctx:
            persist = ctx.enter_context(tc.tile_pool(name="persist", bufs=1))

            ident = persist.tile([128, 128], BF16)
            make_identity(nc, ident)

            ones_f = persist.tile([128, 1], F32)
            nc.vector.memset(ones_f, 1.0)
            ones_col = persist.tile([128, 1], BF16)   # M=1 stationary for denom
            nc.vector.tensor_copy(ones_col, ones_f)
            ones_rowf = persist.tile([1, 128], F32)
            nc.vector.memset(ones_rowf, 1.0)
            ones_row = persist.tile([1, 128], F32R)   # K=1 stationary for broadcast
            nc.vector.tensor_copy(ones_row, ones_rowf)

            # diagonal-block 0/1 masks (r = kb - 4c in 0..3): valid iff sq >= r*128 + sk
            masks = persist.tile([128, 4, 512], BF16)
            mask_f = persist.tile([128, 512], F32)
            for r in range(4):
                nc.vector.memset(mask_f, 1.0)
                nc.gpsimd.affine_select(
                    out=mask_f, in_=mask_f, compare_op=AluOpType.is_ge,
                    fill=0.0, base=-128 * r, pattern=[[1, 512]], channel_multiplier=-1,
                )
                nc.vector.tensor_copy(masks[:, r, :], mask_f)

            eps_t = persist.tile([128, 1], F32)
            nc.vector.memset(eps_t, EPS)

            gains_sb = persist.tile([128, G], F32)
            nc.sync.dma_start(out=gains_sb, in_=gains)
            cos_all = persist.tile([128, NST, G * HALF], F32)
            sin_all = persist.tile([128, NST, G * HALF], F32)
            nc.sync.dma_start(out=cos_all, in_=cos_r)
            nc.sync.dma_start(out=sin_all, in_=sin_r)

            qT_all = persist.tile([128, G, S], BF16)   # [f, g, s]
            kT_all = persist.tile([128, S], BF16)      # [f, s]
            v_all = persist.tile([128, NST, HD], BF16)  # [sk within tile, st, f]
            yT_all = persist.tile([128, G, S], BF16)   # [f, g, s]
            wqkv_sb = persist.tile([128, NDT, FQKV], BF16)
            wpT_sb = persist.tile([128, G, D], BF16)

            xc_pool = ctx.enter_context(tc.tile_pool(name="xc", bufs=3))
            p1 = ctx.enter_context(tc.tile_pool(name="p1", bufs=2))
            p2 = ctx.enter_context(tc.tile_pool(name="p2", bufs=2))
            ps = ctx.enter_context(tc.tile_pool(name="ps", bufs=1, space="PSUM"))

            # x s-tile 0 first so the first matmul isn't gated on all weights
            xcs = []
            xc0 = xc_pool.tile([128, NDT, 128], BF16, tag="xc", name="xc0")
            nc.sync.dma_start(out=xc0, in_=xT_r[:, :, 0:128])
            xcs.append(xc0)
            for dt in range(NDT):
                nc.sync.dma_start(out=wqkv_sb[:, dt, :], in_=wqkv_r[:, dt, :])
            nc.sync.dma_start(out=wpT_sb, in_=wpT_r)

            def transpose_tile(st, q_sc, k_sc):
                # PE identity-transposes into [f, s]; evacuations on ScalarE
                for h in range(G):
                    ptr = ps.tile([128, 128], BF16, tag="optr", bufs=2, name="ptr")
                    nc.tensor.transpose(ptr, q_sc[:, h, :], ident)
                    nc.scalar.copy(qT_all[:, h, st * 128:(st + 1) * 128], ptr)
                ptr = ps.tile([128, 128], BF16, tag="optr", bufs=2, name="ptr")
                nc.tensor.transpose(ptr, k_sc, ident)
                nc.scalar.copy(kT_all[:, st * 128:(st + 1) * 128], ptr)

            def qkv_tile(st, prev):
                xc = xcs[st]
                if st + 1 < NST:
                    nxt = xc_pool.tile([128, NDT, 128], BF16, tag="xc", name="xcn")
                    nc.sync.dma_start(
                        out=nxt, in_=xT_r[:, :, (st + 1) * 128:(st + 2) * 128])
                    xcs.append(nxt)

                psq = ps.tile([128, G * HD], F32, tag="mm512", bufs=3, name="psq")
                pskv = ps.tile([128, 2 * HD], F32, tag="kvdd", bufs=1, name="pskv")
                for dt in range(NDT):
                    nc.tensor.matmul(psq, xc[:, dt, :], wqkv_sb[:, dt, 0:G * HD],
                                     start=(dt == 0), stop=(dt == NDT - 1))
                for dt in range(NDT):
                    nc.tensor.matmul(pskv, xc[:, dt, :], wqkv_sb[:, dt, G * HD:FQKV],
                                     start=(dt == 0), stop=(dt == NDT - 1))
                # previous tile's transposes slot in here: their inputs are
                # ready, so the PE never waits on the rope chain
                if prev is not None:
                    transpose_tile(*prev)

                # rms statistics: Square with free-axis accumulation (ScalarE)
                psq_v = psq.rearrange("p (g d) -> p g d", g=G)
                ms = p1.tile([128, 5], F32, tag="ms")
                sq = p1.tile([128, HD], F32, tag="sq")
                for h in range(G):
                    nc.scalar.activation(sq, psq_v[:, h, :],
                                         mybir.ActivationFunctionType.Square,
                                         accum_out=ms[:, h:h + 1])
                nc.scalar.activation(sq, pskv[:, 0:HD],
                                     mybir.ActivationFunctionType.Square,
                                     accum_out=ms[:, 4:5])
                rstd = p1.tile([128, 5], F32, tag="rstd")
                nc.scalar.activation(rstd, ms, mybir.ActivationFunctionType.Sqrt,
                                     scale=1.0 / HD, bias=eps_t)
                nc.vector.reciprocal(rstd, rstd)
                gsc = p1.tile([128, G], F32, tag="gsc")
                nc.vector.tensor_mul(gsc, rstd[:, 0:4], gains_sb)

                # v: straight evacuation (no norm) on DVE
                nc.vector.tensor_copy(v_all[:, st, :], pskv[:, HD:2 * HD])

                # rope rotation (reads PSUM directly), then pass-through copy
                cos_t = cos_all[:, st, :].rearrange("p (g d) -> p g d", g=G)
                sin_t = sin_all[:, st, :].rearrange("p (g d) -> p g d", g=G)

                q_rot = p1.tile([128, G, HD], F32, tag="q_rot")
                tmp = p1.tile([128, G, HALF], F32, tag="tmp")
                nc.vector.tensor_mul(q_rot[:, :, 0:HALF], psq_v[:, :, 0:HALF], cos_t)
                nc.vector.tensor_mul(tmp, psq_v[:, :, HALF:ROPE], sin_t)
                nc.vector.tensor_add(q_rot[:, :, 0:HALF], q_rot[:, :, 0:HALF], tmp)
                nc.vector.tensor_mul(q_rot[:, :, HALF:ROPE], psq_v[:, :, HALF:ROPE], cos_t)
                nc.vector.tensor_mul(tmp, psq_v[:, :, 0:HALF], sin_t)
                nc.vector.tensor_sub(q_rot[:, :, HALF:ROPE], q_rot[:, :, HALF:ROPE], tmp)
                nc.vector.tensor_copy(q_rot[:, :, ROPE:HD], psq_v[:, :, ROPE:HD])

                k_rot = p1.tile([128, HD], F32, tag="k_rot")
                ktmp = p1.tile([128, HALF], F32, tag="ktmp")
                kc = cos_all[:, st, 0:HALF]
                ks = sin_all[:, st, 0:HALF]
                nc.vector.tensor_mul(k_rot[:, 0:HALF], pskv[:, 0:HALF], kc)
                nc.vector.tensor_mul(ktmp, pskv[:, HALF:ROPE], ks)
                nc.vector.tensor_add(k_rot[:, 0:HALF], k_rot[:, 0:HALF], ktmp)
                nc.vector.tensor_mul(k_rot[:, HALF:ROPE], pskv[:, HALF:ROPE], kc)
                nc.vector.tensor_mul(ktmp, pskv[:, 0:HALF], ks)
                nc.vector.tensor_sub(k_rot[:, HALF:ROPE], k_rot[:, HALF:ROPE], ktmp)
                nc.vector.tensor_copy(k_rot[:, ROPE:HD], pskv[:, ROPE:HD])

                # scale (q by gain*rstd/sqrt(hd), k by rstd) + cast to bf16
                q_sc = p1.tile([128, G, HD], BF16, tag="q_sc")
                k_sc = p1.tile([128, HD], BF16, tag="k_sc")
                for h in range(G):
                    nc.vector.tensor_scalar_mul(q_sc[:, h, :], q_rot[:, h, :],
                                                gsc[:, h:h + 1])
                nc.vector.tensor_scalar_mul(k_sc, k_rot, rstd[:, 4:5])
                return st, q_sc, k_sc

            def attn_group(c, g):
                nkv = 4 * (c + 1)
                qT_c = qT_all[:, g, c * 512:(c + 1) * 512]
                ps_y = ps.tile([128, 512], F32, tag="y", bufs=2, name="ps_y")
                ps_d = ps.tile([1, 512], F32, tag="kvdd", bufs=1, name="ps_d")
                for kb in range(nkv):
                    r = kb - 4 * c
                    off = 128 * r if r > 0 else 0
                    ps_s = ps.tile([128, 512], F32, tag="mm512", bufs=3, name="ps_s")
                    nc.tensor.matmul(ps_s[:, off:512],
                                     kT_all[:, kb * 128:(kb + 1) * 128],
                                     qT_c[:, off:512], start=True, stop=True)
                    t = p2.tile([128, 512], F32, tag="t", name="t")
                    nc.scalar.activation(t[:, off:512], ps_s[:, off:512],
                                         mybir.ActivationFunctionType.Tanh,
                                         scale=1.0 / CAP)
                    p = p2.tile([128, 512], BF16, tag="p", bufs=3, name="p")
                    nc.scalar.activation(p[:, off:512], t[:, off:512],
                                         mybir.ActivationFunctionType.Exp,
                                         scale=CAP)
                    if r >= 0:
                        nc.vector.tensor_mul(p[:, off:512], p[:, off:512],
                                             masks[:, r, off:512])
                    nc.tensor.matmul(ps_y[:, off:512], v_all[:, kb, :],
                                     p[:, off:512],
                                     start=(kb == 0), stop=(kb == nkv - 1))
                    nc.tensor.matmul(ps_d[:, off:512], ones_col, p[:, off:512],
                                     start=(kb == 0), stop=(kb == nkv - 1))
                # denom -> sbuf row -> broadcast to 128 partitions -> recip
                dn = p2.tile([1, 512], F32R, tag="dn", name="dn")
                nc.vector.tensor_copy(dn, ps_d)
                ps_b = ps.tile([128, 512], F32, tag="kvdd", bufs=1, name="ps_b")
                nc.tensor.matmul(ps_b, ones_row, dn, start=True, stop=True)
                recip = p2.tile([128, 512], F32, tag="recip", name="recip")
                nc.vector.reciprocal(recip, ps_b)
                nc.vector.tensor_mul(yT_all[:, g, c * 512:(c + 1) * 512], ps_y, recip)

            def proj_group(st, jc):
                ps_o = ps.tile([128, 512], F32, tag="optr", bufs=2, name="ps_o")
                for g in range(G):
                    nc.tensor.matmul(ps_o,
                                     yT_all[:, g, st * 128:(st + 1) * 128],
                                     wpT_sb[:, g, jc * 512:(jc + 1) * 512],
                                     start=(g == 0), stop=(g == G - 1))
                o_sb = p2.tile([128, 512], F32, tag="o_sb", bufs=3, name="o_sb")
                nc.vector.tensor_copy(o_sb, ps_o)
                nc.sync.dma_start(out=out_r[st][:, jc * 512:(jc + 1) * 512], in_=o_sb)

            prev = None
            for st in range(NST):
                prev = qkv_tile(st, prev)
            transpose_tile(*prev)

            # chunk c-1's output projection interleaves into chunk c's
            # attention groups to fill the scalar-bound stretches
            pending = []
            for c in range(NCH):
                for g in range(G):
                    attn_group(c, g)
                    for _ in range(4):
                        if pending:
                            proj_group(*pending.pop(0))
                for st in range(4 * c, 4 * c + 4):
                    for jc in range(4):
                        pending.append((st, jc))
            while pending:
                proj_group(*pending.pop(0))

    nc.compile()
    return nc


def _host_prep(x, Wq, Wk, Wv, Wproj, q_gain):
    inv_freq = 1.0 / (ROPE_BASE ** (np.arange(0, ROPE, 2, dtype=np.float32) / ROPE))
    t = np.arange(S, dtype=np.float32)
    freqs = np.outer(t, inv_freq).astype(np.float32)  # [S, 16]
    cos = np.cos(freqs).astype(np.float32)
    sin = np.sin(freqs).astype(np.float32)
    cos4 = np.ascontiguousarray(np.tile(cos[:, None, :], (1, G, 1)).reshape(S, G * HALF))
    sin4 = np.ascontiguousarray(np.tile(sin[:, None, :], (1, G, 1)).reshape(S, G * HALF))

    xT = [np.ascontiguousarray(x[b].T).astype(ml_dtypes.bfloat16) for b in range(B)]

    in_maps = []
    for core in range(8):
        b, h = core // HKV, core % HKV
        wqkv = np.ascontiguousarray(
            np.concatenate(
                [Wq[512 * h:512 * h + 512].T,
                 Wk[128 * h:128 * h + 128].T,
                 Wv[128 * h:128 * h + 128].T], axis=1
            )
        ).astype(ml_dtypes.bfloat16)
        wpT = np.ascontiguousarray(
            Wproj[:, 512 * h:512 * h + 512].T).astype(ml_dtypes.bfloat16)
        gains = np.ascontiguousarray(
            np.broadcast_to((q_gain[G * h:G * h + G] / math.sqrt(HD)).astype(np.float32)[None, :],
                            (128, G))
        )
        in_maps.append({
            "xT": xT[b],
            "wqkv": wqkv,
            "wpT": wpT,
            "gains": gains,
            "cos4": cos4,
            "sin4": sin4,
        })
    return in_maps


def kernel(x, Wq, Wk, Wv, Wproj, q_gain, _trace=False):
    x = np.asarray(x, dtype=np.float32)
    Wq = np.asarray(Wq, dtype=np.float32)
    Wk = np.asarray(Wk, dtype=np.float32)
    Wv = np.asarray(Wv, dtype=np.float32)
    Wproj = np.asarray(Wproj, dtype=np.float32)
    q_gain = np.asarray(q_gain, dtype=np.float32)

    if "nc" not in _CACHE:
        _CACHE["nc"] = _build()
    nc = _CACHE["nc"]

    in_maps = _host_prep(x, Wq, Wk, Wv, Wproj, q_gain)
    res = run_bass_kernel_spmd(nc, in_maps, core_ids=list(range(8)), trace=_trace)

    out = np.empty((B, S, D), dtype=np.float32)
    for b in range(B):
        acc = np.zeros((S, D), dtype=np.float64)
        for h in range(HKV):
            acc += res.results[b * HKV + h]["out"]
        out[b] = acc.astype(np.float32)
    if _trace:
        return out, res
    return out


# revision 6
# speedup vs baseline: 1.4574x; 1.2252x over previous
"""Causal self-attention (GQA, partial RoPE, qk rms-norm, logit softcap) on 8 trn2 cores.

Sharding: 8 cores = batch(2) x kv_head(4). Each core computes, for its (b, h):
  - q/k/v projections for its 4 q-heads / 1 kv-head (x @ W.T slices)
  - rms-norm, partial rope, q_gain, causal softcapped attention
  - partial output projection against Wproj columns [512h:512h+512]
Host sums the 4 partials per batch.

All heavy matmuls run in bf16 (halves DMA/SBUF traffic vs f32r at the same
1 cycle/row PE issue rate). Scores are computed transposed (sT[sk, sq]) so
attention @ V needs no transpose of the probabilities; the softmax denominator
comes from a per-kb M=1 ones-matmul accumulated in PSUM.

Engine-budget notes (ACT has ~300ns/instr fixed cost, DVE ~150-250ns):
  - tanh/exp run on kb PAIRS ([128,2,512] PSUM tiles) to halve ACT dispatches;
    the pair sub-regions a diagonal block doesn't own hold bounded garbage
    (|capped logit| <= CAP) that the causal mask multiply zeroes anyway.
  - rms stats: one fused DVE multiply+reduce per head (tensor_tensor_reduce).
  - q/k transposes: PE identity-matmuls into one shared PSUM bank, deferred a
    tile so they never stall on the rope chain; evacuated in 2 ACT copies.
  - softmax normalization uses reciprocal_approx_fast (~18 bits, 5x faster).
  - weights DMA in single descriptors; x s-tile 0 issued first (SP engine
    serializes DMA issue at ~0.76us each).
"""
import math
import numpy as np
from contextlib import ExitStack

import ml_dtypes
import concourse.bass as bass
import concourse.tile as tile
from concourse import bacc, mybir
from concourse.bass_utils import run_bass_kernel_spmd
from concourse.masks import make_identity
from concourse.alu_op_type import AluOpType

F32 = mybir.dt.float32
F32R = mybir.dt.float32r
BF16 = mybir.dt.bfloat16

B = 2
S = 2048
D = 2048
H = 16
HKV = 4
HD = 128
G = 4  # q heads per kv head (= heads per core)
ROPE = 32
HALF = ROPE // 2  # 16
ROPE_BASE = 10000.0
CAP = 30.0
EPS = float(np.finfo(np.float32).eps)
NST = S // 128  # 16 s-tiles
NCH = S // 512  # 4 sq chunks
NDT = D // 128  # 16 d k-tiles
FQKV = G * HD + 2 * HD  # 768

_CACHE = {}


def _build():
    nc = bacc.Bacc("TRN2", target_bir_lowering=False, debug=False)

    xT = nc.dram_tensor("xT", [D, S], BF16, kind="ExternalInput").ap()
    wqkv = nc.dram_tensor("wqkv", [D, FQKV], BF16, kind="ExternalInput").ap()
    wpT = nc.dram_tensor("wpT", [G * HD, D], BF16, kind="ExternalInput").ap()
    gains = nc.dram_tensor("gains", [128, G], F32, kind="ExternalInput").ap()
    cos4 = nc.dram_tensor("cos4", [S, G * HALF], F32, kind="ExternalInput").ap()
    sin4 = nc.dram_tensor("sin4", [S, G * HALF], F32, kind="ExternalInput").ap()
    out = nc.dram_tensor("out", [S, D], F32, kind="ExternalOutput").ap()

    xT_r = xT.rearrange("(dt p) s -> p dt s", p=128)       # [128, 16, 2048]
    wqkv_r = wqkv.rearrange("(dt p) f -> p dt f", p=128)   # [128, 16, 768]
    wpT_r = wpT.rearrange("(g p) j -> p g j", p=128)       # [128, 4, 2048]
    cos_r = cos4.rearrange("(t p) f -> p t f", p=128)      # [128, 16, 64]
    sin_r = sin4.rearrange("(t p) f -> p t f", p=128)
    out_r = out.rearrange("(t p) j -> t p j", p=128)       # [16, 128, 2048]

    with tile.TileContext(nc) as tc:
        with ExitStack() as ctx:
            persist = ctx.enter_context(tc.tile_pool(name="persist", bufs=1))

            ident = persist.tile([128, 128], BF16)
            make_identity(nc, ident)

            ones_f = persist.tile([128, 1], F32)
            nc.vector.memset(ones_f, 1.0)
            ones_col = persist.tile([128, 1], BF16)   # M=1 stationary for denom
            nc.vector.tensor_copy(ones_col, ones_f)
            ones_rowf = persist.tile([1, 128], F32)
            nc.vector.memset(ones_rowf, 1.0)
            ones_row = persist.tile([1, 128], F32R)   # K=1 stationary for broadcast
            nc.vector.tensor_copy(ones_row, ones_rowf)

            # diagonal-block 0/1 masks (r = kb - 4c in 0..3): valid iff sq >= r*128 + sk
            masks = persist.tile([128, 4, 512], BF16)
            mask_f = persist.tile([128, 512], F32)
            for r in range(4):
                nc.vector.memset(mask_f, 1.0)
                nc.gpsimd.affine_select(
                    out=mask_f, in_=mask_f, compare_op=AluOpType.is_ge,
                    fill=0.0, base=-128 * r, pattern=[[1, 512]], channel_multiplier=-1,
                )
                nc.vector.tensor_copy(masks[:, r, :], mask_f)

            eps_t = persist.tile([128, 1], F32)
            nc.vector.memset(eps_t, EPS)

            gains_sb = persist.tile([128, G], F32)
            cos_all = persist.tile([128, NST, G * HALF], F32)
            sin_all = persist.tile([128, NST, G * HALF], F32)

            qT_all = persist.tile([128, G, S], BF16)   # [f, g, s]
            kT_all = persist.tile([128, S], BF16)      # [f, s]
            v_all = persist.tile([128, NST, HD], BF16)  # [sk within tile, st, f]
            yT_all = persist.tile([128, G, S], BF16)   # [f, g, s]
            wqkv_sb = persist.tile([128, NDT, FQKV], BF16)
            wpT_sb = persist.tile([128, G, D], BF16)

            xc_pool = ctx.enter_context(tc.tile_pool(name="xc", bufs=3))
            p1 = ctx.enter_context(tc.tile_pool(name="p1", bufs=2))
            p2 = ctx.enter_context(tc.tile_pool(name="p2", bufs=2))
            ps = ctx.enter_context(tc.tile_pool(name="ps", bufs=1, space="PSUM"))

            # DMA issue order matters: x s-tile 0, then the big weights, then
            # the small constants
            xcs = []
            xc0 = xc_pool.tile([128, NDT, 128], BF16, tag="xc", name="xc0")
            nc.sync.dma_start(out=xc0, in_=xT_r[:, :, 0:128])
            nc.sync.dma_start(out=wqkv_sb, in_=wqkv_r)
            nc.sync.dma_start(out=wpT_sb, in_=wpT_r)
            nc.sync.dma_start(out=gains_sb, in_=gains)
            nc.sync.dma_start(out=cos_all, in_=cos_r)
            nc.sync.dma_start(out=sin_all, in_=sin_r)
            xcs.append(xc0)

            def transpose_tile(st, q_sc, k_sc):
                # 5 PE identity-transposes into one PSUM bank; 2 ACT evacs
                ptr = ps.tile([128, 5 * HD], BF16, tag="ptr", bufs=1, name="ptr")
                for h in range(G):
                    nc.tensor.transpose(ptr[:, h * HD:(h + 1) * HD],
                                        q_sc[:, h, :], ident)
                nc.tensor.transpose(ptr[:, 4 * HD:5 * HD], k_sc, ident)
                nc.scalar.copy(
                    qT_all[:, :, st * 128:(st + 1) * 128],
                    ptr[:, 0:4 * HD].rearrange("p (g d) -> p g d", g=G))
                nc.scalar.copy(kT_all[:, st * 128:(st + 1) * 128],
                               ptr[:, 4 * HD:5 * HD])

            def qkv_tile(st, prev):
                xc = xcs[st]
                if st + 1 < NST:
                    nxt = xc_pool.tile([128, NDT, 128], BF16, tag="xc", name="xcn")
                    nc.sync.dma_start(
                        out=nxt, in_=xT_r[:, :, (st + 1) * 128:(st + 2) * 128])
                    xcs.append(nxt)

                psq = ps.tile([128, G * HD], F32, tag="ymm", bufs=2, name="psq")
                pskv = ps.tile([128, 2 * HD], F32, tag="kvdd", bufs=1, name="pskv")
                for dt in range(NDT):
                    nc.tensor.matmul(psq, xc[:, dt, :], wqkv_sb[:, dt, 0:G * HD],
                                     start=(dt == 0), stop=(dt == NDT - 1))
                for dt in range(NDT):
                    nc.tensor.matmul(pskv, xc[:, dt, :], wqkv_sb[:, dt, G * HD:FQKV],
                                     start=(dt == 0), stop=(dt == NDT - 1))
                # previous tile's transposes slot in here: their inputs are
                # ready, so the PE never waits on the rope chain
                if prev is not None:
                    transpose_tile(*prev)

                # rms statistics: one wide ACT square per q/k, DVE reduces
                psq_v = psq.rearrange("p (g d) -> p g d", g=G)
                ms = p1.tile([128, 5], F32, tag="ms")
                q2 = p1.tile([128, G * HD], F32, tag="q2")
                k2 = p1.tile([128, HD], F32, tag="k2")
                nc.scalar.activation(q2, psq, mybir.ActivationFunctionType.Square)
                nc.scalar.activation(k2, pskv[:, 0:HD],
                                     mybir.ActivationFunctionType.Square)
                nc.vector.reduce_sum(ms[:, 0:4],
                                     q2.rearrange("p (g d) -> p g d", g=G),
                                     axis=mybir.AxisListType.X)
                nc.vector.reduce_sum(ms[:, 4:5], k2, axis=mybir.AxisListType.X)
                rstd = p1.tile([128, 5], F32, tag="rstd")
                nc.scalar.activation(rstd, ms, mybir.ActivationFunctionType.Sqrt,
                                     scale=1.0 / HD, bias=eps_t)
                nc.vector.reciprocal(rstd, rstd)
                gsc = p1.tile([128, G], F32, tag="gsc")
                nc.vector.tensor_mul(gsc, rstd[:, 0:4], gains_sb)

                # v: straight evacuation (no norm) on DVE
                nc.vector.tensor_copy(v_all[:, st, :], pskv[:, HD:2 * HD])

                # rope rotation (reads PSUM directly), then pass-through copy
                cos_t = cos_all[:, st, :].rearrange("p (g d) -> p g d", g=G)
                sin_t = sin_all[:, st, :].rearrange("p (g d) -> p g d", g=G)

                q_rot = p1.tile([128, G, HD], F32, tag="q_rot")
                tmp = p1.tile([128, G, HALF], F32, tag="tmp")
                nc.vector.tensor_mul(q_rot[:, :, 0:HALF], psq_v[:, :, 0:HALF], cos_t)
                nc.vector.tensor_mul(tmp, psq_v[:, :, HALF:ROPE], sin_t)
                nc.vector.tensor_add(q_rot[:, :, 0:HALF], q_rot[:, :, 0:HALF], tmp)
                nc.vector.tensor_mul(q_rot[:, :, HALF:ROPE], psq_v[:, :, HALF:ROPE], cos_t)
                nc.vector.tensor_mul(tmp, psq_v[:, :, 0:HALF], sin_t)
                nc.vector.tensor_sub(q_rot[:, :, HALF:ROPE], q_rot[:, :, HALF:ROPE], tmp)
                nc.vector.tensor_copy(q_rot[:, :, ROPE:HD], psq_v[:, :, ROPE:HD])

                k_rot = p1.tile([128, HD], F32, tag="k_rot")
                ktmp = p1.tile([128, HALF], F32, tag="ktmp")
                kc = cos_all[:, st, 0:HALF]
                ks = sin_all[:, st, 0:HALF]
                nc.vector.tensor_mul(k_rot[:, 0:HALF], pskv[:, 0:HALF], kc)
                nc.vector.tensor_mul(ktmp, pskv[:, HALF:ROPE], ks)
                nc.vector.tensor_add(k_rot[:, 0:HALF], k_rot[:, 0:HALF], ktmp)
                nc.vector.tensor_mul(k_rot[:, HALF:ROPE], pskv[:, HALF:ROPE], kc)
                nc.vector.tensor_mul(ktmp, pskv[:, 0:HALF], ks)
                nc.vector.tensor_sub(k_rot[:, HALF:ROPE], k_rot[:, HALF:ROPE], ktmp)
                nc.vector.tensor_copy(k_rot[:, ROPE:HD], pskv[:, ROPE:HD])

                # scale (q by gain*rstd/sqrt(hd), k by rstd) + cast to bf16
                q_sc = p1.tile([128, G, HD], BF16, tag="q_sc")
                k_sc = p1.tile([128, HD], BF16, tag="k_sc")
                for h in range(G):
                    nc.vector.tensor_scalar_mul(q_sc[:, h, :], q_rot[:, h, :],
                                                gsc[:, h:h + 1])
                nc.vector.tensor_scalar_mul(k_sc, k_rot, rstd[:, 4:5])
                return st, q_sc, k_sc

            def attn_group(c, g):
                nkv = 4 * (c + 1)
                qT_c = qT_all[:, g, c * 512:(c + 1) * 512]
                ps_y = ps.tile([128, 512], F32, tag="ymm", bufs=2, name="ps_y")
                ps_d = ps.tile([1, 512], F32, tag="kvdd", bufs=1, name="ps_d")
                for kb0 in range(0, nkv, 2):
                    rr = kb0 - 4 * c  # r of the pair's first block
                    # tanh/exp width for the pair; garbage sub-regions are
                    # bounded by the softcap and zeroed by the mask multiply
                    poff = 256 if rr == 2 else 0
                    ps_s = ps.tile([128, 2, 512], F32, tag="ss", bufs=2, name="ps_s")
                    for i in range(2):
                        r = rr + i
                        off = 128 * r if r > 0 else 0
                        nc.tensor.matmul(ps_s[:, i, off:512],
                                         kT_all[:, (kb0 + i) * 128:(kb0 + i + 1) * 128],
                                         qT_c[:, off:512], start=True, stop=True)
                    t = p2.tile([128, 2, 512], F32, tag="t", name="t")
                    nc.scalar.activation(t[:, :, poff:512], ps_s[:, :, poff:512],
                                         mybir.ActivationFunctionType.Tanh,
                                         scale=1.0 / CAP)
                    p = p2.tile([128, 2, 512], BF16, tag="p", bufs=3, name="p")
                    nc.scalar.activation(p[:, :, poff:512], t[:, :, poff:512],
                                         mybir.ActivationFunctionType.Exp,
                                         scale=CAP)
                    if rr >= 0:
                        nc.vector.tensor_mul(p[:, :, poff:512], p[:, :, poff:512],
                                             masks[:, rr:rr + 2, poff:512])
                    for i in range(2):
                        kb = kb0 + i
                        r = rr + i
                        off = 128 * r if r > 0 else 0
                        nc.tensor.matmul(ps_y[:, off:512], v_all[:, kb, :],
                                         p[:, i, off:512],
                                         start=(kb == 0), stop=(kb == nkv - 1))
                        nc.tensor.matmul(ps_d[:, off:512], ones_col,
                                         p[:, i, off:512],
                                         start=(kb == 0), stop=(kb == nkv - 1))
                # denom -> sbuf row -> broadcast to 128 partitions -> recip
                dn = p2.tile([1, 512], F32R, tag="dn", name="dn")
                nc.vector.tensor_copy(dn, ps_d)
                ps_b = ps.tile([128, 512], F32, tag="kvdd", bufs=1, name="ps_b")
                nc.tensor.matmul(ps_b, ones_row, dn, start=True, stop=True)
                recip = p2.tile([128, 512], F32, tag="recip", name="recip")
                nc.vector.reciprocal_approx_fast(out=recip, in_=ps_b)
                nc.vector.tensor_mul(yT_all[:, g, c * 512:(c + 1) * 512], ps_y, recip)

            def proj_group(st, jc, n):
                ps_o = ps.tile([128, 512], F32, tag="ymm", bufs=2, name="ps_o")
                for g in range(G):
                    nc.tensor.matmul(ps_o,
                                     yT_all[:, g, st * 128:(st + 1) * 128],
                                     wpT_sb[:, g, jc * 512:(jc + 1) * 512],
                                     start=(g == 0), stop=(g == G - 1))
                o_sb = p2.tile([128, 512], F32, tag="o_sb", bufs=3, name="o_sb")
                # alternate evacuations between ACT and DVE to balance load
                if n % 2 == 0:
                    nc.scalar.copy(o_sb, ps_o)
                else:
                    nc.vector.tensor_copy(o_sb, ps_o)
                nc.sync.dma_start(out=out_r[st][:, jc * 512:(jc + 1) * 512], in_=o_sb)

            prev = None
            for st in range(NST):
                prev = qkv_tile(st, prev)
            transpose_tile(*prev)

            # chunk c-1's output projection interleaves into chunk c's
            # attention groups to fill the scalar-bound stretches
            pending = []
            nproj = 0
            for c in range(NCH):
                for g in range(G):
                    attn_group(c, g)
                    for _ in range(4):
                        if pending:
                            proj_group(*pending.pop(0), nproj)
                            nproj += 1
                for st in range(4 * c, 4 * c + 4):
                    for jc in range(4):
                        pending.append((st, jc))
            while pending:
                proj_group(*pending.pop(0), nproj)
                nproj += 1

    nc.compile()
    return nc


def _host_prep(x, Wq, Wk, Wv, Wproj, q_gain):
    inv_freq = 1.0 / (ROPE_BASE ** (np.arange(0, ROPE, 2, dtype=np.float32) / ROPE))
    t = np.arange(S, dtype=np.float32)
    freqs = np.outer(t, inv_freq).astype(np.float32)  # [S, 16]
    cos = np.cos(freqs).astype(np.float32)
    sin = np.sin(freqs).astype(np.float32)
    cos4 = np.ascontiguousarray(np.tile(cos[:, None, :], (1, G, 1)).reshape(S, G * HALF))
    sin4 = np.ascontiguousarray(np.tile(sin[:, None, :], (1, G, 1)).reshape(S, G * HALF))

    xT = [np.ascontiguousarray(x[b].T).astype(ml_dtypes.bfloat16) for b in range(B)]

    in_maps = []
    for core in range(8):
        b, h = core // HKV, core % HKV
        wqkv = np.ascontiguousarray(
            np.concatenate(
                [Wq[512 * h:512 * h + 512].T,
                 Wk[128 * h:128 * h + 128].T,
                 Wv[128 * h:128 * h + 128].T], axis=1
            )
        ).astype(ml_dtypes.bfloat16)
        wpT = np.ascontiguousarray(
            Wproj[:, 512 * h:512 * h + 512].T).astype(ml_dtypes.bfloat16)
        gains = np.ascontiguousarray(
            np.broadcast_to((q_gain[G * h:G * h + G] / math.sqrt(HD)).astype(np.float32)[None, :],
                            (128, G))
        )
        in_maps.append({
            "xT": xT[b],
            "wqkv": wqkv,
            "wpT": wpT,
            "gains": gains,
            "cos4": cos4,
            "sin4": sin4,
        })
    return in_maps


def kernel(x, Wq, Wk, Wv, Wproj, q_gain, _trace=False):
    x = np.asarray(x, dtype=np.float32)
    Wq = np.asarray(Wq, dtype=np.float32)
    Wk = np.asarray(Wk, dtype=np.float32)
    Wv = np.asarray(Wv, dtype=np.float32)
    Wproj = np.asarray(Wproj, dtype=np.float32)
    q_gain = np.asarray(q_gain, dtype=np.float32)

    if "nc" not in _CACHE:
        _CACHE["nc"] = _build()
    nc = _CACHE["nc"]

    in_maps = _host_prep(x, Wq, Wk, Wv, Wproj, q_gain)
    res = run_bass_kernel_spmd(nc, in_maps, core_ids=list(range(8)), trace=_trace)

    out = np.empty((B, S, D), dtype=np.float32)
    for b in range(B):
        acc = np.zeros((S, D), dtype=np.float64)
        for h in range(HKV):
            acc += res.results[b * HKV + h]["out"]
        out[b] = acc.astype(np.float32)
    if _trace:
        return out, res
    return out
